# revision 24
# baseline (speedup 1.0000x reference)
"""Bass/Trainium2 kernel for nn_Attention_7816840478804 (ragged bag-attention).

Reference computation:
    att[i]   = <x[i], rel_weight[label[i]]>                       # [N]
    e[i]     = softmax of att within each bag (segment)           # [N]
    repre[b] = sum_{i in b} e[i] * x[i] / sum_{i in b} e[i]       # [B, D]
    logits   = repre @ rel_weight.T + bias                        # [B, C]

Key algebraic fusion used here: matmul distributes over the weighted sum, so
    logits[b] = (sum_i e_i * att_all[i, :]) / (sum_i e_i) + bias
with att_all = x @ rel_weight.T  [N, C].  x is therefore read exactly once and
the bag pooling happens on the tiny [N, 53] matrix.  Softmax stabilization
(max subtraction) is dropped: it cancels exactly, and |att| < ~10 here so
exp() cannot overflow.

Sharding: sentences are split across 8 cores on bag boundaries (2048 bags
per core, host-side searchsorted), padded to a common block count so all
cores run the same SPMD graph.  Per-sentence bag-slot indices are
host-precomputed *data*, keeping the instruction stream static.
"""

import sys

sys.path.insert(0, "/opt/trn_rl_repo")

import numpy as np

N_CORES = 8
B_TOTAL = 16384
BPC = B_TOTAL // N_CORES  # 2048 bags per core
C = 53
D = 768
NCH = D // 128  # 6 contraction chunks
BLK = 512  # sentences per block (one att matmul)
TILE = 128  # sentences per pooling tile
BAGS_PER_BLK = 64  # expected bags covered by one block (512 sent / 8 per bag)


# ---------------------------------------------------------------------------
# Host-side packing
# ---------------------------------------------------------------------------

def _pack(x, label, segment_ids, rel_weight, bias):
    """Shard + lay out inputs for the device graph. Returns (in_maps, meta)."""
    x = np.ascontiguousarray(np.asarray(x, dtype=np.float32))
    label = np.asarray(label).astype(np.int64)
    seg = np.asarray(segment_ids).astype(np.int64)
    rw = np.asarray(rel_weight, dtype=np.float32)
    bs = np.asarray(bias, dtype=np.float32)
    n = x.shape[0]

    edges = np.searchsorted(seg, np.arange(0, B_TOTAL + 1, BPC), side="left")
    lens = np.diff(edges)
    padn = int(np.ceil(lens.max() / BLK) * BLK)
    nblk = padn // BLK
    nt = padn // TILE

    # slot_raw = seg_local - 64*block; find required symmetric-ish padding
    lo, hi = 0, 0
    per_core = []
    for c in range(N_CORES):
        s, e = int(edges[c]), int(edges[c + 1])
        seg_local = seg[s:e] - c * BPC
        g = np.arange(e - s) // BLK
        slot_raw = seg_local - BAGS_PER_BLK * g
        if len(slot_raw):
            lo = min(lo, int(slot_raw.min()))
            hi = max(hi, int(slot_raw.max()))
        per_core.append((s, e, slot_raw))
    padb = max(-lo, hi - (BAGS_PER_BLK - 1), 8)
    padb = int(np.ceil(padb / 8) * 8)
    w = BAGS_PER_BLK + 2 * padb
    assert w <= 512

    in_maps = []
    for c in range(N_CORES):
        s, e, slot_raw = per_core[c]
        ln = e - s
        xs = np.zeros((padn, D), dtype=np.float32)
        xs[:ln] = x[s:e]
        # (block, partition=dchunk-row, chunk, col) = x[g*512+j, ch*128+p]
        xp = np.ascontiguousarray(
            xs.reshape(nblk, BLK, NCH, 128).transpose(0, 3, 2, 1)
        ).reshape(nblk, 128, NCH * BLK)

        lab = np.zeros(padn, dtype=np.float32)
        lab[:ln] = label[s:e].astype(np.float32)
        lab_t = np.ascontiguousarray(lab.reshape(nt, TILE).T)

        slot = np.full(padn, -1.0, dtype=np.float32)
        slot[:ln] = (slot_raw + padb).astype(np.float32)
        assert slot[:ln].min() >= 0 and slot[:ln].max() < w
        slot_t = np.ascontiguousarray(slot.reshape(nt, TILE).T)

        in_maps.append({
            "xin": xp,
            "labT": lab_t,
            "slotT": slot_t,
            "wtp": np.ascontiguousarray(
                rw.T.reshape(NCH, 128, C).transpose(1, 0, 2)
            ).reshape(128, NCH * C),
            "iota53": np.tile(np.arange(C, dtype=np.float32), (128, 1)),
            "iotaw": np.tile(np.arange(w, dtype=np.float32), (128, 1)),
            "eye": np.eye(C, dtype=np.float32),
            "onesr": np.ones((1, C + 1), dtype=np.float32),
            "biasc": np.concatenate([np.zeros((1, 1), np.float32), bs.reshape(C, 1)]),
        })

    meta = {"nblk": nblk, "nt": nt, "w": w, "padb": padb, "edges": edges}
    return in_maps, meta


def _numpy_emulate(in_maps, meta):
    """Pure-numpy emulation of the device graph (layout validation)."""
    nblk, w, padb = meta["nblk"], meta["w"], meta["padb"]
    accw = BAGS_PER_BLK * nblk + 2 * padb
    outs = []
    for m in in_maps:
        wt = m["wtp"].reshape(128, NCH, C)
        acc = np.zeros((C + 1, accw), dtype=np.float32)
        for g in range(nblk):
            xsb = m["xin"][g].reshape(128, NCH, BLK)
            att = np.zeros((C, BLK), dtype=np.float32)
            for ch in range(NCH):
                att += wt[:, ch, :].T @ xsb[:, ch, :]
            for t in range(4):
                col = g * 4 + t
                at = att[:, t * TILE:(t + 1) * TILE].T  # [128, 53]
                at1 = np.concatenate([np.ones((TILE, 1), np.float32), at], 1)
                lab = m["labT"][:, col]
                asel = (at * (m["iota53"] == lab[:, None])).sum(1)
                ev = np.exp(asel)
                sel = (m["iotaw"] == m["slotT"][:, col][:, None]) * ev[:, None]
                acc[:, BAGS_PER_BLK * g:BAGS_PER_BLK * g + w] += at1.T @ sel
        den = np.maximum(acc[0, padb:padb + BPC], 1e-30)
        outs.append(acc[1:, padb:padb + BPC] / den + m["biasc"][1:])
    return np.concatenate([o.T for o in outs], 0)


# ---------------------------------------------------------------------------
# Device graph
# ---------------------------------------------------------------------------

_GRAPH_CACHE = {}


def _build(nblk, w, padb):
    key = (nblk, w, padb)
    if key in _GRAPH_CACHE:
        return _GRAPH_CACHE[key]

    import concourse.bacc as bacc
    import concourse.mybir as mybir
    from concourse import tile

    f32 = mybir.dt.float32
    f32r = mybir.dt.float32r
    Alu = mybir.AluOpType
    Act = mybir.ActivationFunctionType
    nt = nblk * 4
    accw = BAGS_PER_BLK * nblk + 2 * padb

    nc = bacc.Bacc("TRN2", target_bir_lowering=False, debug=False)
    xin = nc.dram_tensor("xin", [nblk, 128, NCH * BLK], f32r, kind="ExternalInput").ap()
    labT = nc.dram_tensor("labT", [128, nt], f32, kind="ExternalInput").ap()
    slotT = nc.dram_tensor("slotT", [128, nt], f32, kind="ExternalInput").ap()
    wtp = nc.dram_tensor("wtp", [128, NCH * C], f32r, kind="ExternalInput").ap()
    iota53 = nc.dram_tensor("iota53", [128, C], f32, kind="ExternalInput").ap()
    iotaw = nc.dram_tensor("iotaw", [128, w], f32, kind="ExternalInput").ap()
    eye = nc.dram_tensor("eye", [C, C], f32, kind="ExternalInput").ap()
    onesr = nc.dram_tensor("onesr", [1, C + 1], f32, kind="ExternalInput").ap()
    biasc = nc.dram_tensor("biasc", [C + 1, 1], f32, kind="ExternalInput").ap()
    out_t = nc.dram_tensor("out", [C + 1, BPC], f32, kind="ExternalOutput").ap()

    with tile.TileContext(nc) as tc:
        with (
            tc.tile_pool(name="const", bufs=1) as cpool,
            tc.tile_pool(name="accp", bufs=1) as accpool,
            tc.tile_pool(name="xp", bufs=4) as xpool,
            tc.tile_pool(name="attp", bufs=3) as apool,
            tc.tile_pool(name="small", bufs=8) as spool,
            tc.tile_pool(name="ep", bufs=2) as epool,
            tc.tile_pool(name="ps_att", bufs=2, space="PSUM") as ps_att,
            tc.tile_pool(name="ps_tr", bufs=2, space="PSUM") as ps_tr,
            tc.tile_pool(name="ps_num", bufs=2, space="PSUM") as ps_num,
        ):
            wt_sb = cpool.tile([128, NCH * C], f32r, tag="wt")
            nc.sync.dma_start(wt_sb, wtp)
            iota53_sb = cpool.tile([128, C], f32, tag="i53")
            nc.sync.dma_start(iota53_sb, iota53)
            iotaw_sb = cpool.tile([128, w], f32, tag="iw")
            nc.sync.dma_start(iotaw_sb, iotaw)
            eye_sb = cpool.tile([C, C], f32, tag="eye")
            nc.sync.dma_start(eye_sb, eye)
            onesr_sb = cpool.tile([1, C + 1], f32, tag="onesr")
            nc.sync.dma_start(onesr_sb, onesr)
            biasc_sb = cpool.tile([C + 1, 1], f32, tag="bias")
            nc.sync.dma_start(biasc_sb, biasc)
            labT_sb = cpool.tile([128, nt], f32, tag="lab")
            nc.sync.dma_start(labT_sb, labT)
            slotT_sb = cpool.tile([128, nt], f32, tag="slot")
            nc.sync.dma_start(slotT_sb, slotT)

            acc = accpool.tile([C + 1, accw], f32, tag="acc")
            nc.vector.memset(acc, 0.0)

            for g in range(nblk):
                x_sb = xpool.tile([128, NCH * BLK], f32r, tag="x")
                nc.sync.dma_start(x_sb, xin[g])

                aps = ps_att.tile([C, BLK], f32, tag="aps")
                for ch in range(NCH):
                    nc.tensor.matmul(
                        aps,
                        wt_sb[:, ch * C:(ch + 1) * C],
                        x_sb[:, ch * BLK:(ch + 1) * BLK],
                        start=(ch == 0),
                        stop=(ch == NCH - 1),
                    )
                att_row = apool.tile([C, BLK], f32, tag="attrow")
                nc.scalar.copy(att_row, aps)

                nps = ps_num.tile([C + 1, w], f32, tag="nps")
                for t in range(4):
                    col = g * 4 + t
                    trp = ps_tr.tile([128, C], f32, tag="trp")
                    nc.tensor.transpose(
                        trp, att_row[:, t * TILE:(t + 1) * TILE], eye_sb
                    )
                    # at1 col 0 = ones (denominator), cols 1..53 = att
                    at1 = spool.tile([128, C + 1], f32, tag="at1")
                    nc.scalar.copy(at1[:, 1:C + 1], trp)
                    nc.vector.memset(at1[:, 0:1], 1.0)
                    # asel = sum((iota53 == label) * att, free axis)
                    scr = spool.tile([128, C], f32, tag="scr")
                    asel = spool.tile([128, 1], f32, tag="asel")
                    nc.vector.scalar_tensor_tensor(
                        scr,
                        iota53_sb,
                        labT_sb[:, col:col + 1],
                        at1[:, 1:C + 1],
                        Alu.is_equal,
                        Alu.mult,
                        accum_out=asel,
                    )
                    ev = spool.tile([128, 1], f32, tag="ev")
                    nc.scalar.activation(ev, asel, Act.Exp)
                    sel = spool.tile([128, w], f32, tag="sel")
                    nc.vector.tensor_scalar(
                        sel,
                        iotaw_sb,
                        slotT_sb[:, col:col + 1],
                        ev,
                        Alu.is_equal,
                        Alu.mult,
                    )
                    nc.tensor.matmul(nps, at1, sel, start=(t == 0), stop=(t == 3))

                off = BAGS_PER_BLK * g
                nc.vector.scalar_tensor_tensor(
                    acc[:, off:off + w],
                    nps,
                    0.0,
                    acc[:, off:off + w],
                    Alu.bypass,
                    Alu.add,
                )

            # epilogue: logits = num / max(den, eps) + bias, per 512-bag chunk
            for p in range(BPC // BLK):
                sl = slice(padb + p * BLK, padb + (p + 1) * BLK)
                den_b = ps_att.tile([C + 1, BLK], f32, tag="aps")
                # broadcast den row across 53 partitions via rank-1 matmul
                nc.tensor.matmul(
                    den_b,
                    onesr_sb,
                    acc[0:1, sl],
                    start=True,
                    stop=True,
                )
                den_sb = epool.tile([C + 1, BLK], f32, tag="densb")
                nc.vector.tensor_scalar(den_sb, den_b, 1e-30, None, Alu.max)
                rec = epool.tile([C + 1, BLK], f32, tag="rec")
                nc.vector.reciprocal(rec, den_sb)
                quot = epool.tile([C + 1, BLK], f32, tag="quot")
                nc.vector.scalar_tensor_tensor(
                    quot, acc[0:C + 1, sl], 0.0, rec, Alu.bypass, Alu.mult
                )
                ob = epool.tile([C + 1, BLK], f32, tag="ob")
                nc.scalar.activation(ob, quot, Act.Identity, bias=biasc_sb)
                nc.sync.dma_start(out_t[:, p * BLK:(p + 1) * BLK], ob)

    nc.compile()
    _GRAPH_CACHE[key] = nc
    return nc


# ---------------------------------------------------------------------------
# Entry point
# ---------------------------------------------------------------------------

_last_results = None


def _install_ntff_hook():
    """Provide antenv.axon_hooks (missing in this image) from trn_boot."""
    try:
        from antenv import axon_hooks  # noqa: F401
        return
    except ImportError:
        pass
    import types

    import antenv
    from trn_agent_boot.trn_boot import _ntff_profile_via_ctypes

    hook = _ntff_profile_via_ctypes("/opt/axon/libaxon_pjrt.so")
    m = types.ModuleType("antenv.axon_hooks")
    m.get_axon_ntff_profile_hook = lambda: hook
    m.set_axon_ntff_profile_hook = lambda h: None
    sys.modules["antenv.axon_hooks"] = m
    antenv.axon_hooks = m


def kernel(x, label, segment_ids, rel_weight, bias):
    import concourse.bass_utils as bu
    from concourse.bass_utils import run_bass_kernel_spmd

    in_maps, meta = _pack(x, label, segment_ids, rel_weight, bias)
    nc = _build(meta["nblk"], meta["w"], meta["padb"])

    global _last_results
    import os

    trace = bool(os.environ.get("KERNEL_TRACE"))
    tmpdir = None
    if trace:
        _install_ntff_hook()
        bu.upload_artifacts = lambda d: d  # no bucket in this container
        tmpdir = os.environ.get("KERNEL_TRACE_DIR")
    res = run_bass_kernel_spmd(
        nc, in_maps, core_ids=list(range(N_CORES)), trace=trace, tmpdir=tmpdir
    )
    _last_results = res
    out = np.empty((B_TOTAL, C), dtype=np.float32)
    for c in range(N_CORES):
        out[c * BPC:(c + 1) * BPC] = res.results[c]["out"][1:].T
    return out
